# revision 9
# baseline (speedup 1.0000x reference)
"""LIF (leaky integrate-and-fire with hard reset) spike-train kernel for TRN2.

Problem: x [32, 4096, 256] f32; scan over last (time) axis:
    u = u*0.125 + x_t ; s = (u >= 1) ; u = (1-s)*u
Output: spikes [32, 4096, 256] f32 (0.0/1.0).

Strategy: data-parallel over the 131072 independent neurons across 8 cores
(16384 each).  Per core, neurons live as [128 partitions x 128 columns]; the
time recurrence runs as a fully unrolled instruction loop:
  W1 (DVE scalar_tensor_tensor): u_pre = (u * tau) + x_t
  W2 (ACT Sign):                 out_t = sign(1 - u_pre)   in {-1,0,+1}
  W3 (DVE scalar_tensor_tensor): u     = (u_pre < 1) * u_pre
Host decodes spikes = (out <= 0), which matches the >= threshold exactly.

Time-stagger: T=256 is split into STAG_B blocks computed concurrently
(independent free-dim columns b*128+g, all holding the same 16384 neurons at
different time offsets), which multiplies the per-instruction free-dim by
STAG_B and amortizes the ~150-cycle fixed DVE instruction overhead.  Each
block is warmed up for WARM steps from u=0: tau^WARM = 8^-WARM makes the
warm-start state bit-identical to the true state (the error decays 8x per
step; a divergent warmup spike also re-decays, so only flips in the last ~8
warmup steps could matter, with probability ~8^-(WARM-8) per neuron-block).

Block b's warmup inputs are block b-1's inputs at steps L-WARM..L-1, i.e. an
affine column shift in the same x tensor — so the input needs NO duplication:
warmup runs as narrower instructions (blocks 1..B-1, block 0 starts at true
t=0 with u=0) reading a shifted slice.  All x chunks stay resident in SBUF
(no tile recycling -> no slot-WAR waits on DMAs).

Host pre-arranges input per core as xs[p, step, b, g] = x[p*128+g, 64b+step]
(partition-contiguous DMA, contiguous [128, FD] compute slices) and decodes
os[p, j, b, g] -> spikes[neuron = p*128+g, t = L*b + j].
"""

import numpy as np

# ---- problem constants (hardcoded; kernel.py must be self-contained) ----
B_, N_, T_ = 32, 4096, 256
NCORES = 8
NEUR = B_ * N_              # 131072 neurons total
NPC = NEUR // NCORES        # 16384 neurons per core
TAU = 0.125
VTH = 1.0

# ---- kernel configuration ----
STAG_B = 4        # number of staggered time blocks (1 = plain sequential)
WARM = 16         # warmup steps per block (block 0 needs none)
TC = 2            # time-steps per input DMA chunk (must divide L_)
ODMA_K = 2        # output steps per output-DMA
OUT_MODE = "act_sign_i8"   # how the spike output is produced
COMPUTE = 1       # 0 = DMA-only variant (roofline measurement)
UPRE_BUFS = 2     # buffers for the u_pre scratch pool
LOOP_K = 0        # benchmark-only: repeat the whole body K times (tc.For_i)
W3G = 0           # columns [FD-W3G:FD] whose W3 reset runs on GPSIMD (2 ops)
PE_W1 = 0         # 1 = integrate on TensorE (2 accumulating identity matmuls
                  # into PSUM, per column group), ACT copies PSUM->SBUF,
                  # spike compare split ACT(grp0)/DVE(grp1), DVE does resets
                  # 2 = TensorE integrate, DVE reset STRAIGHT FROM PSUM (no
                  # ACT copy), ACT Sign spike straight from PSUM
NGRP = 2          # column groups for the PE_W1 pipeline
MM_DT = "f32r"    # matmul operand mode for PE_W1=2: f32r | f32 | bf16w

L_ = T_ // STAG_B           # block length (= steps with output)
FD = STAG_B * 128           # free dim of main compute instructions
WFD = (STAG_B - 1) * 128    # free dim of warmup instructions

_cache = {}


def _build_nc():
    import concourse.mybir as mybir
    from concourse.bacc import Bacc
    from concourse.tile import TileContext

    # Bacc (not plain Bass): its compile() pass splits multi-semaphore waits
    # into event-semaphore instructions — walrus rejects >1 wait per inst.
    nc = Bacc(None, target_bir_lowering=False)
    f32 = mybir.dt.float32
    Alu = mybir.AluOpType
    Act = mybir.ActivationFunctionType

    assert L_ % TC == 0
    n_chunks = L_ // TC

    xs = nc.dram_tensor("xs", [128, L_, FD], f32, kind="ExternalInput")
    wid = None
    if PE_W1:
        # [tau*I | I] stationary weights for the two accumulating matmuls
        wdt = mybir.dt.bfloat16 if MM_DT == "bf16w" else f32
        wid = nc.dram_tensor("wid", [128, 256], wdt, kind="ExternalInput")
    if OUT_MODE == "act_sign_i8":
        odt = mybir.dt.int8
    elif OUT_MODE == "act_sign_bf16":
        odt = mybir.dt.bfloat16
    else:
        odt = mybir.dt.uint8
    osd = nc.dram_tensor("os", [128, L_, FD], odt, kind="ExternalOutput")

    # chunks containing the warmup columns (steps L_-WARM .. L_-1) load first
    wc0 = (L_ - WARM) // TC if STAG_B > 1 and WARM > 0 else n_chunks
    load_order = list(range(wc0, n_chunks)) + list(range(0, wc0))

    with TileContext(nc) as tc:
        with (
            tc.tile_pool(name="state", bufs=1) as spool,
            tc.tile_pool(name="xw", bufs=1) as xpool,
            tc.tile_pool(name="ow", bufs=1) as opool,
            tc.tile_pool(name="upre", bufs=UPRE_BUFS) as upool,
        ):
            u = spool.tile([128, FD], f32)
            nc.vector.memset(u[:, :], 0.0)

            xw = {
                ci: xpool.tile(
                    [128, TC, FD], f32, tag=f"xw{ci}", name=f"xw{ci}"
                )
                for ci in load_order
            }
            ow = {
                ci: opool.tile(
                    [128, TC, FD], odt, tag=f"ow{ci}", name=f"ow{ci}"
                )
                for ci in range(n_chunks)
            }

            import contextlib

            pe = None
            if PE_W1:
                with tc.tile_pool(name="pe", bufs=1) as wpool, \
                     tc.tile_pool(name="psum", bufs=2, space="PSUM") as ppool:
                    wdt = mybir.dt.bfloat16 if MM_DT == "bf16w" else f32
                    wsb = wpool.tile([128, 256], wdt, name="wsb")
                    nc.sync.dma_start(out=wsb[:, :], in_=wid[:, :])
                    pe = (wsb, ppool)
                    loop_cm = (
                        tc.For_i(0, LOOP_K, 1)
                        if LOOP_K else contextlib.nullcontext()
                    )
                    with loop_cm:
                        _emit_body(
                            nc, tc, mybir, xs, osd, xw, ow, u, upool,
                            n_chunks, pe,
                        )
            else:
                loop_cm = (
                    tc.For_i(0, LOOP_K, 1) if LOOP_K else contextlib.nullcontext()
                )
                with loop_cm:
                    _emit_body(
                        nc, tc, mybir, xs, osd, xw, ow, u, upool, n_chunks, None
                    )
    nc.finalize()
    return nc


def _emit_body(nc, tc, mybir, xs, osd, xw, ow, u, upool, n_chunks, pe=None):
    f32 = mybir.dt.float32
    Alu = mybir.AluOpType
    Act = mybir.ActivationFunctionType
    load_order = list(xw.keys())
    if True:
        if True:
            for ci in load_order:  # noqa: E501 (indent kept from inline version)
                nc.sync.dma_start(
                    out=xw[ci][:, :, :], in_=xs[:, ci * TC : (ci + 1) * TC, :]
                )

            if COMPUTE and pe is not None and PE_W1 == 2:
                _emit_body_pe2(nc, tc, mybir, osd, xw, ow, u, upool, pe)
                return

            if COMPUTE and STAG_B > 1 and WARM > 0:
                # Warmup: blocks 1..B-1 (state cols 128:FD) read block b-1's
                # columns at steps L_-WARM+tw (cols 0:WFD), starting from u=0.
                for tw in range(WARM):
                    col = L_ - WARM + tw
                    ci, cl = divmod(col, TC)
                    upw = upool.tile([128, WFD], f32, tag="upw")
                    nc.vector.scalar_tensor_tensor(
                        out=upw[:, :], in0=u[:, 128:FD], scalar=TAU,
                        in1=xw[ci][:, cl, 0:WFD],
                        op0=Alu.mult, op1=Alu.add,
                    )
                    nc.vector.scalar_tensor_tensor(
                        out=u[:, 128:FD], in0=upw[:, :], scalar=VTH,
                        in1=upw[:, :],
                        op0=Alu.is_lt, op1=Alu.mult,
                    )

            if COMPUTE and pe is not None:
                wsb, ppool = pe
                GW = FD // NGRP
                for step in range(L_):
                    ci, cl = divmod(step, TC)
                    for g in range(NGRP):
                        gs0, gs1 = g * GW, (g + 1) * GW
                        pp = ppool.tile(
                            [128, GW], f32, tag=f"pp{g}", name=f"pp{g}_{step}"
                        )
                        nc.tensor.matmul(
                            pp[:, :], wsb[:, 0:128], u[:, gs0:gs1],
                            start=True, stop=False,
                        )
                        nc.tensor.matmul(
                            pp[:, :], wsb[:, 128:256],
                            xw[ci][:, cl, gs0:gs1],
                            start=False, stop=True,
                        )
                        upg = upool.tile(
                            [128, GW], f32, tag=f"upg{g}", name=f"upg{g}_{step}"
                        )
                        nc.scalar.copy(out=upg[:, :], in_=pp[:, :])
                        if g == 0:
                            # spike via ACT Sign straight from PSUM
                            nc.scalar.activation(
                                out=ow[ci][:, cl, gs0:gs1], in_=pp[:, :],
                                func=Act.Sign, bias=1.0, scale=-1.0,
                            )
                        else:
                            # spike via DVE is_ge on the SBUF copy (1/0 i8)
                            nc.vector.tensor_scalar(
                                ow[ci][:, cl, gs0:gs1], upg[:, :], VTH,
                                None, Alu.is_ge,
                            )
                        nc.vector.scalar_tensor_tensor(
                            out=u[:, gs0:gs1], in0=upg[:, :], scalar=VTH,
                            in1=upg[:, :],
                            op0=Alu.is_lt, op1=Alu.mult,
                        )
                    if (step + 1) % ODMA_K == 0 or step == L_ - 1:
                        g1_ = step + 1
                        g0_ = g1_ - (g1_ % ODMA_K or ODMA_K)
                        c0, l0 = divmod(g0_, TC)
                        nc.sync.dma_start(
                            out=osd[:, g0_:g1_, :],
                            in_=ow[c0][:, l0 : l0 + (g1_ - g0_), :],
                        )
            elif COMPUTE:
                A = FD - W3G
                for step in range(L_):
                    ci, cl = divmod(step, TC)
                    up = upool.tile([128, FD], f32, tag="up")
                    nc.vector.scalar_tensor_tensor(
                        out=up[:, :], in0=u[:, :], scalar=TAU,
                        in1=xw[ci][:, cl, :],
                        op0=Alu.mult, op1=Alu.add,
                    )
                    # sign(1 - u_pre): +1 no spike, -1/0 spike.
                    # (bias=1.0 has a registered const AP; -1.0 does not.)
                    nc.scalar.activation(
                        out=ow[ci][:, cl, :], in_=up[:, :],
                        func=Act.Sign, bias=1.0, scale=-1.0,
                    )
                    nc.vector.scalar_tensor_tensor(
                        out=u[:, 0:A], in0=up[:, 0:A], scalar=VTH,
                        in1=up[:, 0:A],
                        op0=Alu.is_lt, op1=Alu.mult,
                    )
                    if W3G:
                        mg = upool.tile([128, W3G], f32, tag="mg")
                        nc.gpsimd.tensor_scalar(
                            mg[:, :], up[:, A:FD], VTH, None, Alu.is_lt
                        )
                        nc.gpsimd.tensor_tensor(
                            u[:, A:FD], mg[:, :], up[:, A:FD], Alu.mult
                        )
                    if (step + 1) % ODMA_K == 0 or step == L_ - 1:
                        g1 = step + 1
                        g0 = g1 - (g1 % ODMA_K or ODMA_K)
                        c0, l0 = divmod(g0, TC)
                        nc.sync.dma_start(
                            out=osd[:, g0:g1, :],
                            in_=ow[c0][:, l0 : l0 + (g1 - g0), :],
                        )
            else:
                for ci in range(n_chunks):
                    nc.vector.memset(ow[ci][:, :, :], 0)
                    nc.sync.dma_start(
                        out=osd[:, ci * TC : (ci + 1) * TC, :],
                        in_=ow[ci][:, :, :],
                    )


def _emit_body_pe2(nc, tc, mybir, osd, xw, ow, u, upool, pe):
    """TensorE integrate -> PSUM; DVE reset PSUM->SBUF; ACT Sign PSUM->i8.

    Per column group g and step t (groups are independent pipelines):
      MM1: pp_g  = (tau*I).T @ u_g          (start)
      MM2: pp_g += I.T @ x_t_g              (stop)
      ACT: ow_g  = Sign(1 - pp_g)           (i8 spike encoding)
      DVE: u_g   = (pp_g < 1) * pp_g        (hard reset)
    Warmup steps (state cols 128:FD reading shifted x cols 0:WFD) run the
    same MM/DVE pattern without the ACT spike.
    """
    f32 = mybir.dt.float32
    Alu = mybir.AluOpType
    Act = mybir.ActivationFunctionType
    wsb, ppool = pe
    mmdt = {
        "f32r": mybir.dt.float32r,
        "f32": mybir.dt.float32,
        "bf16w": mybir.dt.float32r,
    }[MM_DT]

    def mm_cast(ap):
        return ap.bitcast(mmdt) if mmdt != f32 else ap

    def w_cast(ap):
        if MM_DT == "bf16w":
            return ap
        return mm_cast(ap)

    def emit_step(cols, x_ap_fn, spike_ci_cl, tag):
        """cols: list of (g0, g1) column groups."""
        for gi, (g0, g1) in enumerate(cols):
            gw = g1 - g0
            pp = ppool.tile([128, gw], f32, tag=f"pp{tag}{gi}")
            nc.tensor.matmul(
                pp[:, :], w_cast(wsb[:, 0:128]), mm_cast(u[:, g0:g1]),
                start=True, stop=False,
            )
            nc.tensor.matmul(
                pp[:, :], w_cast(wsb[:, 128:256]), mm_cast(x_ap_fn(g0, g1)),
                start=False, stop=True,
            )
            if spike_ci_cl is not None:
                ci, cl = spike_ci_cl
                nc.scalar.activation(
                    out=ow[ci][:, cl, g0:g1], in_=pp[:, :],
                    func=Act.Sign, bias=1.0, scale=-1.0,
                )
            nc.vector.scalar_tensor_tensor(
                out=u[:, g0:g1], in0=pp[:, :], scalar=VTH, in1=pp[:, :],
                op0=Alu.is_lt, op1=Alu.mult,
            )

    if STAG_B > 1 and WARM > 0:
        gw = WFD // NGRP
        wcols = [
            (128 + i * gw, 128 + (i + 1) * gw if i < NGRP - 1 else FD)
            for i in range(NGRP)
        ]
        for tw in range(WARM):
            col = L_ - WARM + tw
            ci, cl = divmod(col, TC)
            emit_step(
                wcols,
                lambda g0, g1, ci=ci, cl=cl: xw[ci][:, cl, g0 - 128 : g1 - 128],
                None,
                "w",
            )

    gw = FD // NGRP
    mcols = [(i * gw, (i + 1) * gw) for i in range(NGRP)]
    for step in range(L_):
        ci, cl = divmod(step, TC)
        emit_step(
            mcols,
            lambda g0, g1, ci=ci, cl=cl: xw[ci][:, cl, g0:g1],
            (ci, cl),
            "m",
        )
        if (step + 1) % ODMA_K == 0 or step == L_ - 1:
            g1_ = step + 1
            g0_ = g1_ - (g1_ % ODMA_K or ODMA_K)
            c0, l0 = divmod(g0_, TC)
            nc.sync.dma_start(
                out=osd[:, g0_:g1_, :],
                in_=ow[c0][:, l0 : l0 + (g1_ - g0_), :],
            )


def _prep_core_input(xc):
    """xc: [128, 128, 256] (p, g, t) f32 -> xs [128, L_, STAG_B, 128]."""
    # xs[p, step, b, g] = xc[p, g, L_*b + step]
    return np.ascontiguousarray(
        xc.reshape(128, 128, STAG_B, L_).transpose(0, 3, 2, 1)
    )


def _extra_inputs():
    if not PE_W1:
        return {}
    w = np.zeros((128, 256), dtype=np.float32)
    idx = np.arange(128)
    w[idx, idx] = TAU
    w[idx, 128 + idx] = 1.0
    if MM_DT == "bf16w":
        import ml_dtypes
        w = w.astype(ml_dtypes.bfloat16)
    return {"wid": w}


def _decode_core_output(o):
    """o: [128, L_, FD] (or flat) -> spikes [16384, 256] f32."""
    o4 = np.asarray(o).reshape(128, L_, STAG_B, 128)
    if PE_W1 == 1:
        # group 0 cols: ACT Sign encoding; group 1 cols: DVE is_ge (1/0)
        bsplit = (FD // NGRP) // 128
        sp = np.empty(o4.shape, dtype=bool)
        sp[:, :, :bsplit] = o4[:, :, :bsplit] <= 0
        sp[:, :, bsplit:] = o4[:, :, bsplit:] != 0
    elif OUT_MODE.startswith("act_sign"):
        sp = (np.asarray(o4, dtype=np.float32) <= 0.0)
    else:
        sp = np.asarray(o4) != 0
    # [p, j, b, g] -> [p, g, b, j] -> [16384, 256]
    return (
        sp.transpose(0, 3, 2, 1).reshape(NPC, T_).astype(np.float32)
    )


def kernel(x, _trace=False):
    from concourse.bass_utils import run_bass_kernel_spmd

    x = np.ascontiguousarray(np.asarray(x), dtype=np.float32)
    assert x.shape == (B_, N_, T_)
    xf = x.reshape(NEUR, T_)

    in_maps = []
    for c in range(NCORES):
        xc = xf[c * NPC : (c + 1) * NPC].reshape(128, 128, T_)
        m = {"xs": _prep_core_input(xc)}
        m.update(_extra_inputs())
        in_maps.append(m)

    if "nc" not in _cache:
        _cache["nc"] = _build_nc()
    nc = _cache["nc"]

    res = run_bass_kernel_spmd(
        nc, in_maps, core_ids=list(range(NCORES)), trace=_trace
    )
    kernel.last_result = res

    out = np.empty((NEUR, T_), dtype=np.float32)
    for c in range(NCORES):
        out[c * NPC : (c + 1) * NPC] = _decode_core_output(res.results[c]["os"])
    return out.reshape(B_, N_, T_)


kernel.last_result = None



# revision 14
# speedup vs baseline: 2.1097x; 2.1097x over previous
"""LIF (leaky integrate-and-fire with hard reset) spike-train kernel for TRN2.

Problem: x [32, 4096, 256] f32; scan over last (time) axis:
    u = u*0.125 + x_t ; s = (u >= 1) ; u = (1-s)*u
Output: spikes [32, 4096, 256] f32 (0.0/1.0).

Strategy: data-parallel over the 131072 independent neurons across 8 cores
(16384 each).  Per core, neurons live as [128 partitions x 128 columns]; the
time recurrence runs as a fully unrolled instruction loop:
  W1 (DVE scalar_tensor_tensor): u_pre = (u * tau) + x_t
  W2 (ACT Sign):                 out_t = sign(1 - u_pre)   in {-1,0,+1}
  W3 (DVE scalar_tensor_tensor): u     = (u_pre < 1) * u_pre
Host decodes spikes = (out <= 0), which matches the >= threshold exactly.

Time-stagger: T=256 is split into STAG_B blocks computed concurrently
(independent free-dim columns b*128+g, all holding the same 16384 neurons at
different time offsets), which multiplies the per-instruction free-dim by
STAG_B and amortizes the ~150-cycle fixed DVE instruction overhead.  Each
block is warmed up for WARM steps from u=0: tau^WARM = 8^-WARM makes the
warm-start state bit-identical to the true state (the error decays 8x per
step; a divergent warmup spike also re-decays, so only flips in the last ~8
warmup steps could matter, with probability ~8^-(WARM-8) per neuron-block).

Block b's warmup inputs are block b-1's inputs at steps L-WARM..L-1, i.e. an
affine column shift in the same x tensor — so the input needs NO duplication:
warmup runs as narrower instructions (blocks 1..B-1, block 0 starts at true
t=0 with u=0) reading a shifted slice.  All x chunks stay resident in SBUF
(no tile recycling -> no slot-WAR waits on DMAs).

Host pre-arranges input per core as xs[p, step, b, g] = x[p*128+g, 64b+step]
(partition-contiguous DMA, contiguous [128, FD] compute slices) and decodes
os[p, j, b, g] -> spikes[neuron = p*128+g, t = L*b + j].
"""

import numpy as np

# ---- problem constants (hardcoded; kernel.py must be self-contained) ----
B_, N_, T_ = 32, 4096, 256
NCORES = 8
NEUR = B_ * N_              # 131072 neurons total
NPC = NEUR // NCORES        # 16384 neurons per core
TAU = 0.125
VTH = 1.0

# ---- kernel configuration ----
STAG_B = 4        # number of staggered time blocks (1 = plain sequential)
WARM = 16         # warmup steps per block (block 0 needs none)
TC = 2            # time-steps per input DMA chunk (must divide L_)
ODMA_K = 2        # output steps per output-DMA
OUT_MODE = "act_sign_i8"   # how the spike output is produced
COMPUTE = 1       # 0 = DMA-only variant (roofline measurement)
UPRE_BUFS = 2     # buffers for the u_pre scratch pool
LOOP_K = 0        # benchmark-only: repeat the whole body K times (tc.For_i)
W3G = 0           # columns [FD-W3G:FD] whose W3 reset runs on GPSIMD (2 ops)
PE_W1 = 0         # 1 = integrate on TensorE (2 accumulating identity matmuls
                  # into PSUM, per column group), ACT copies PSUM->SBUF,
                  # spike compare split ACT(grp0)/DVE(grp1), DVE does resets
                  # 2 = TensorE integrate, DVE reset STRAIGHT FROM PSUM (no
                  # ACT copy), ACT Sign spike straight from PSUM
NGRP = 2          # column groups for the PE_W1 pipeline
MM_DT = "f32r"    # matmul operand mode for PE_W1=2: f32r | f32 | bf16w

L_ = T_ // STAG_B           # block length (= steps with output)
FD = STAG_B * 128           # free dim of main compute instructions
WFD = (STAG_B - 1) * 128    # free dim of warmup instructions

_cache = {}


def _build_nc():
    import concourse.mybir as mybir
    from concourse.bacc import Bacc
    from concourse.tile import TileContext

    # Bacc (not plain Bass): its compile() pass splits multi-semaphore waits
    # into event-semaphore instructions — walrus rejects >1 wait per inst.
    nc = Bacc(None, target_bir_lowering=False)
    f32 = mybir.dt.float32
    Alu = mybir.AluOpType
    Act = mybir.ActivationFunctionType

    assert L_ % TC == 0
    n_chunks = L_ // TC

    bf16 = mybir.dt.bfloat16
    f32r = mybir.dt.float32r
    if PE_W1 == 3:
        # x shipped as an exact-sum bf16 pair (hi + lo); 4 B/elem like f32
        xs = nc.dram_tensor("xsh", [128, L_, FD], bf16, kind="ExternalInput")
        xsl = nc.dram_tensor("xsl", [128, L_, FD], bf16, kind="ExternalInput")
    else:
        xs = nc.dram_tensor("xs", [128, L_, FD], f32, kind="ExternalInput")
        xsl = None
    wid = None
    wid2 = None
    if PE_W1 == 3:
        # tau*I as fp32r (tau exactly representable), I as bf16
        wid = nc.dram_tensor("wtau", [128, 128], f32r, kind="ExternalInput")
        wid2 = nc.dram_tensor("wone", [128, 128], bf16, kind="ExternalInput")
    elif PE_W1:
        # [tau*I | I] stationary weights for the two accumulating matmuls
        wdt = mybir.dt.bfloat16 if MM_DT == "bf16w" else f32
        wid = nc.dram_tensor("wid", [128, 256], wdt, kind="ExternalInput")
    if OUT_MODE == "act_sign_i8":
        odt = mybir.dt.int8
    elif OUT_MODE == "act_sign_bf16":
        odt = mybir.dt.bfloat16
    else:
        odt = mybir.dt.uint8
    osd = nc.dram_tensor("os", [128, L_, FD], odt, kind="ExternalOutput")

    # chunks containing the warmup columns (steps L_-WARM .. L_-1) load first
    wc0 = (L_ - WARM) // TC if STAG_B > 1 and WARM > 0 else n_chunks
    load_order = list(range(wc0, n_chunks)) + list(range(0, wc0))

    with TileContext(nc) as tc:
        with (
            tc.tile_pool(name="state", bufs=1) as spool,
            tc.tile_pool(name="xw", bufs=1) as xpool,
            tc.tile_pool(name="ow", bufs=1) as opool,
            tc.tile_pool(name="upre", bufs=UPRE_BUFS) as upool,
        ):
            u = spool.tile([128, FD], f32r if PE_W1 == 3 else f32)
            nc.vector.memset(u[:, :], 0.0)

            xdt = bf16 if PE_W1 == 3 else f32
            xw = {
                ci: xpool.tile(
                    [128, TC, FD], xdt, tag=f"xw{ci}", name=f"xw{ci}"
                )
                for ci in load_order
            }
            xwl = None
            if PE_W1 == 3:
                xwl = {
                    ci: xpool.tile(
                        [128, TC, FD], bf16, tag=f"xwl{ci}", name=f"xwl{ci}"
                    )
                    for ci in load_order
                }
            ow = {
                ci: opool.tile(
                    [128, TC, FD], odt, tag=f"ow{ci}", name=f"ow{ci}"
                )
                for ci in range(n_chunks)
            }

            import contextlib

            pe = None
            if PE_W1:
                with tc.tile_pool(name="pe", bufs=1) as wpool, \
                     tc.tile_pool(name="psum", bufs=2, space="PSUM") as ppool:
                    wdt = mybir.dt.bfloat16 if MM_DT == "bf16w" else f32
                    wsb = wpool.tile([128, 256], wdt, name="wsb")
                    nc.sync.dma_start(out=wsb[:, :], in_=wid[:, :])
                    pe = (wsb, ppool)
                    loop_cm = (
                        tc.For_i(0, LOOP_K, 1)
                        if LOOP_K else contextlib.nullcontext()
                    )
                    with loop_cm:
                        _emit_body(
                            nc, tc, mybir, xs, osd, xw, ow, u, upool,
                            n_chunks, pe,
                        )
            else:
                loop_cm = (
                    tc.For_i(0, LOOP_K, 1) if LOOP_K else contextlib.nullcontext()
                )
                with loop_cm:
                    _emit_body(
                        nc, tc, mybir, xs, osd, xw, ow, u, upool, n_chunks, None
                    )
    nc.finalize()
    return nc


def _emit_body(nc, tc, mybir, xs, osd, xw, ow, u, upool, n_chunks, pe=None):
    f32 = mybir.dt.float32
    Alu = mybir.AluOpType
    Act = mybir.ActivationFunctionType
    load_order = list(xw.keys())
    if True:
        if True:
            for ci in load_order:  # noqa: E501 (indent kept from inline version)
                nc.sync.dma_start(
                    out=xw[ci][:, :, :], in_=xs[:, ci * TC : (ci + 1) * TC, :]
                )

            if COMPUTE and pe is not None and PE_W1 == 2:
                _emit_body_pe2(nc, tc, mybir, osd, xw, ow, u, upool, pe)
                return

            if COMPUTE and STAG_B > 1 and WARM > 0:
                # Warmup: blocks 1..B-1 (state cols 128:FD) read block b-1's
                # columns at steps L_-WARM+tw (cols 0:WFD), starting from u=0.
                for tw in range(WARM):
                    col = L_ - WARM + tw
                    ci, cl = divmod(col, TC)
                    upw = upool.tile([128, WFD], f32, tag="upw")
                    nc.vector.scalar_tensor_tensor(
                        out=upw[:, :], in0=u[:, 128:FD], scalar=TAU,
                        in1=xw[ci][:, cl, 0:WFD],
                        op0=Alu.mult, op1=Alu.add,
                    )
                    nc.vector.scalar_tensor_tensor(
                        out=u[:, 128:FD], in0=upw[:, :], scalar=VTH,
                        in1=upw[:, :],
                        op0=Alu.is_lt, op1=Alu.mult,
                    )

            if COMPUTE and pe is not None:
                wsb, ppool = pe
                GW = FD // NGRP
                for step in range(L_):
                    ci, cl = divmod(step, TC)
                    for g in range(NGRP):
                        gs0, gs1 = g * GW, (g + 1) * GW
                        pp = ppool.tile(
                            [128, GW], f32, tag=f"pp{g}", name=f"pp{g}_{step}"
                        )
                        nc.tensor.matmul(
                            pp[:, :], wsb[:, 0:128], u[:, gs0:gs1],
                            start=True, stop=False,
                        )
                        nc.tensor.matmul(
                            pp[:, :], wsb[:, 128:256],
                            xw[ci][:, cl, gs0:gs1],
                            start=False, stop=True,
                        )
                        upg = upool.tile(
                            [128, GW], f32, tag=f"upg{g}", name=f"upg{g}_{step}"
                        )
                        nc.scalar.copy(out=upg[:, :], in_=pp[:, :])
                        if g == 0:
                            # spike via ACT Sign straight from PSUM
                            nc.scalar.activation(
                                out=ow[ci][:, cl, gs0:gs1], in_=pp[:, :],
                                func=Act.Sign, bias=1.0, scale=-1.0,
                            )
                        else:
                            # spike via DVE is_ge on the SBUF copy (1/0 i8)
                            nc.vector.tensor_scalar(
                                ow[ci][:, cl, gs0:gs1], upg[:, :], VTH,
                                None, Alu.is_ge,
                            )
                        nc.vector.scalar_tensor_tensor(
                            out=u[:, gs0:gs1], in0=upg[:, :], scalar=VTH,
                            in1=upg[:, :],
                            op0=Alu.is_lt, op1=Alu.mult,
                        )
                    if (step + 1) % ODMA_K == 0 or step == L_ - 1:
                        g1_ = step + 1
                        g0_ = g1_ - (g1_ % ODMA_K or ODMA_K)
                        c0, l0 = divmod(g0_, TC)
                        nc.sync.dma_start(
                            out=osd[:, g0_:g1_, :],
                            in_=ow[c0][:, l0 : l0 + (g1_ - g0_), :],
                        )
            elif COMPUTE:
                A = FD - W3G
                for step in range(L_):
                    ci, cl = divmod(step, TC)
                    up = upool.tile([128, FD], f32, tag="up")
                    nc.vector.scalar_tensor_tensor(
                        out=up[:, :], in0=u[:, :], scalar=TAU,
                        in1=xw[ci][:, cl, :],
                        op0=Alu.mult, op1=Alu.add,
                    )
                    # sign(1 - u_pre): +1 no spike, -1/0 spike.
                    # (bias=1.0 has a registered const AP; -1.0 does not.)
                    nc.scalar.activation(
                        out=ow[ci][:, cl, :], in_=up[:, :],
                        func=Act.Sign, bias=1.0, scale=-1.0,
                    )
                    nc.vector.scalar_tensor_tensor(
                        out=u[:, 0:A], in0=up[:, 0:A], scalar=VTH,
                        in1=up[:, 0:A],
                        op0=Alu.is_lt, op1=Alu.mult,
                    )
                    if W3G:
                        mg = upool.tile([128, W3G], f32, tag="mg")
                        nc.gpsimd.tensor_scalar(
                            mg[:, :], up[:, A:FD], VTH, None, Alu.is_lt
                        )
                        nc.gpsimd.tensor_tensor(
                            u[:, A:FD], mg[:, :], up[:, A:FD], Alu.mult
                        )
                    if (step + 1) % ODMA_K == 0 or step == L_ - 1:
                        g1 = step + 1
                        g0 = g1 - (g1 % ODMA_K or ODMA_K)
                        c0, l0 = divmod(g0, TC)
                        nc.sync.dma_start(
                            out=osd[:, g0:g1, :],
                            in_=ow[c0][:, l0 : l0 + (g1 - g0), :],
                        )
            else:
                for ci in range(n_chunks):
                    nc.vector.memset(ow[ci][:, :, :], 0)
                    nc.sync.dma_start(
                        out=osd[:, ci * TC : (ci + 1) * TC, :],
                        in_=ow[ci][:, :, :],
                    )


def _emit_body_pe2(nc, tc, mybir, osd, xw, ow, u, upool, pe):
    """TensorE integrate -> PSUM; DVE reset PSUM->SBUF; ACT Sign PSUM->i8.

    Per column group g and step t (groups are independent pipelines):
      MM1: pp_g  = (tau*I).T @ u_g          (start)
      MM2: pp_g += I.T @ x_t_g              (stop)
      ACT: ow_g  = Sign(1 - pp_g)           (i8 spike encoding)
      DVE: u_g   = (pp_g < 1) * pp_g        (hard reset)
    Warmup steps (state cols 128:FD reading shifted x cols 0:WFD) run the
    same MM/DVE pattern without the ACT spike.
    """
    f32 = mybir.dt.float32
    Alu = mybir.AluOpType
    Act = mybir.ActivationFunctionType
    wsb, ppool = pe
    mmdt = {
        "f32r": mybir.dt.float32r,
        "f32": mybir.dt.float32,
        "bf16w": mybir.dt.float32r,
    }[MM_DT]

    def mm_cast(ap):
        return ap.bitcast(mmdt) if mmdt != f32 else ap

    def w_cast(ap):
        if MM_DT == "bf16w":
            return ap
        return mm_cast(ap)

    def emit_step(cols, x_ap_fn, spike_ci_cl, tag):
        """cols: list of (g0, g1) column groups."""
        for gi, (g0, g1) in enumerate(cols):
            gw = g1 - g0
            pp = ppool.tile([128, gw], f32, tag=f"pp{tag}{gi}")
            nc.tensor.matmul(
                pp[:, :], w_cast(wsb[:, 0:128]), mm_cast(u[:, g0:g1]),
                start=True, stop=False,
            )
            nc.tensor.matmul(
                pp[:, :], w_cast(wsb[:, 128:256]), mm_cast(x_ap_fn(g0, g1)),
                start=False, stop=True,
            )
            nc.vector.scalar_tensor_tensor(
                out=u[:, g0:g1], in0=pp[:, :], scalar=VTH, in1=pp[:, :],
                op0=Alu.is_lt, op1=Alu.mult,
            )
            if spike_ci_cl is not None:
                ci, cl = spike_ci_cl
                nc.scalar.activation(
                    out=ow[ci][:, cl, g0:g1], in_=pp[:, :],
                    func=Act.Sign, bias=1.0, scale=-1.0,
                )

    if STAG_B > 1 and WARM > 0:
        gw = WFD // NGRP
        wcols = [
            (128 + i * gw, 128 + (i + 1) * gw if i < NGRP - 1 else FD)
            for i in range(NGRP)
        ]
        for tw in range(WARM):
            col = L_ - WARM + tw
            ci, cl = divmod(col, TC)
            emit_step(
                wcols,
                lambda g0, g1, ci=ci, cl=cl: xw[ci][:, cl, g0 - 128 : g1 - 128],
                None,
                "w",
            )

    gw = FD // NGRP
    mcols = [(i * gw, (i + 1) * gw) for i in range(NGRP)]
    for step in range(L_):
        ci, cl = divmod(step, TC)
        emit_step(
            mcols,
            lambda g0, g1, ci=ci, cl=cl: xw[ci][:, cl, g0:g1],
            (ci, cl),
            "m",
        )
        if (step + 1) % ODMA_K == 0 or step == L_ - 1:
            g1_ = step + 1
            g0_ = g1_ - (g1_ % ODMA_K or ODMA_K)
            c0, l0 = divmod(g0_, TC)
            nc.sync.dma_start(
                out=osd[:, g0_:g1_, :],
                in_=ow[c0][:, l0 : l0 + (g1_ - g0_), :],
            )


def _prep_core_input(xc):
    """xc: [128, 128, 256] (p, g, t) f32 -> xs [128, L_, STAG_B, 128]."""
    # xs[p, step, b, g] = xc[p, g, L_*b + step]
    return np.ascontiguousarray(
        xc.reshape(128, 128, STAG_B, L_).transpose(0, 3, 2, 1)
    )


def _extra_inputs():
    if not PE_W1:
        return {}
    w = np.zeros((128, 256), dtype=np.float32)
    idx = np.arange(128)
    w[idx, idx] = TAU
    w[idx, 128 + idx] = 1.0
    if MM_DT == "bf16w":
        import ml_dtypes
        w = w.astype(ml_dtypes.bfloat16)
    return {"wid": w}


def _decode_core_output(o):
    """o: [128, L_, FD] (or flat) -> spikes [16384, 256] f32."""
    o4 = np.asarray(o).reshape(128, L_, STAG_B, 128)
    if PE_W1 == 1:
        # group 0 cols: ACT Sign encoding; group 1 cols: DVE is_ge (1/0)
        bsplit = (FD // NGRP) // 128
        sp = np.empty(o4.shape, dtype=bool)
        sp[:, :, :bsplit] = o4[:, :, :bsplit] <= 0
        sp[:, :, bsplit:] = o4[:, :, bsplit:] != 0
    elif OUT_MODE.startswith("act_sign"):
        sp = (np.asarray(o4, dtype=np.float32) <= 0.0)
    else:
        sp = np.asarray(o4) != 0
    # [p, j, b, g] -> [p, g, b, j] -> [16384, 256]
    return (
        sp.transpose(0, 3, 2, 1).reshape(NPC, T_).astype(np.float32)
    )


def kernel(x, _trace=False):
    from concourse.bass_utils import run_bass_kernel_spmd

    x = np.ascontiguousarray(np.asarray(x), dtype=np.float32)
    assert x.shape == (B_, N_, T_)
    xf = x.reshape(NEUR, T_)

    in_maps = []
    for c in range(NCORES):
        xc = xf[c * NPC : (c + 1) * NPC].reshape(128, 128, T_)
        m = {"xs": _prep_core_input(xc)}
        m.update(_extra_inputs())
        in_maps.append(m)

    if "nc" not in _cache:
        _cache["nc"] = _build_nc()
    nc = _cache["nc"]

    res = run_bass_kernel_spmd(
        nc, in_maps, core_ids=list(range(NCORES)), trace=_trace
    )
    kernel.last_result = res

    out = np.empty((NEUR, T_), dtype=np.float32)
    for c in range(NCORES):
        out[c * NPC : (c + 1) * NPC] = _decode_core_output(res.results[c]["os"])
    return out.reshape(B_, N_, T_)


kernel.last_result = None

